# revision 37
# baseline (speedup 1.0000x reference)
"""Trainium2 Bass kernel for CGRE-style ragged bag attention pooling + classifier.

Computation (per reference):
    seg[i]   : bag of sentence i (contiguous ragged scopes)
    s[i]     = X[i] . Constraints[X_Rel[seg[i]]]
    w[i]     = softmax of s within bag (numerically stabilized per bag)
    bag[b]   = sum_{i in b} w[i] * X[i]
    out      = bag @ W.T + b

Strategy: 8-way data parallel over contiguous bag ranges (1024 bags/core).
One shared SPMD Bass program; all per-core raggedness (scope boundaries,
bag windows, gather offsets) is passed as per-core *data* (index tensors),
never baked into the program.

Per core, sentences are processed in 128-row tiles, 4 tiles per "group".
Each group owns a 128-bag window starting at the bag of its first sentence
(windows of adjacent groups overlap; straddling bags get partial sums from
both groups, combined on the host from the per-group [128, R] outputs).

Engine plan (per 128-sentence tile, ~68 tiles/core):
  PE     : conper one-hot gather matmul (1024cy) + transposed pooling into
           bagT [c,w] chunks (1024cy) + m-gather (130cy) + classifier
           (800cy/group) ~= 990ns warm
  DVE    : score dot chunk [P,640] vs PSUM conper + ohp spread [P,128]
  GpSimd : score dot chunk [P,384] + s_lin gather (SWDGE) + masked max add
  Scalar : exp / copies / per-group bagT->SBUF cast
  softmax max/denом run on a DRAM-gathered [bag, LW] layout exactly like
  the reference's segment ops (s_lin roundtrip, pipelined 2 groups deep).

Precision: fp16 operands for matmuls / elementwise, fp32 PSUM accumulation,
per-bag max quantized once to bf16 and used consistently on numerator and
denominator so the quantization cancels.
"""

import numpy as np
from contextlib import ExitStack

import ml_dtypes

import concourse.bass as bass
import concourse.tile as tile
from concourse import bacc, mybir
from concourse.bass import IndirectOffsetOnAxis
from concourse.bass_utils import run_bass_kernel_spmd
from concourse.masks import make_identity
from concourse.tile import add_dep_helper

NCORES = 8
P = 128
TPG = 4          # tiles per group (512 sentences)
F32 = mybir.dt.float32
F16 = mybir.dt.float16
BF16 = mybir.dt.bfloat16
I32 = mybir.dt.int32

CSPL = 640       # dot split: DVE takes [0:CSPL], gpsimd takes [CSPL:C]
DEBUG = False


# ----------------------------------------------------------------------------
# Host-side preparation
# ----------------------------------------------------------------------------

def _prep(X, Constraints, W, b, X_Scope, X_Rel):
    N, C = X.shape
    R = Constraints.shape[0]
    B = X_Scope.shape[0]
    assert B % NCORES == 0
    NB_LOC = B // NCORES

    starts = np.asarray(X_Scope[:, 0], dtype=np.int64)
    ends = np.asarray(X_Scope[:, 1], dtype=np.int64)
    lens = ends - starts
    seg = np.searchsorted(starts, np.arange(N), side="right") - 1
    rel_s = np.asarray(X_Rel, dtype=np.int64)[seg]          # relation per sentence

    LW = int(max(64, ((lens.max() + 31) // 32) * 32))        # gather row width

    core_b0 = [k * NB_LOC for k in range(NCORES)]
    core_s0 = [int(starts[b0]) for b0 in core_b0]
    core_s1 = [int(ends[b0 + NB_LOC - 1]) for b0 in core_b0]
    cnts = [s1 - s0 for s0, s1 in zip(core_s0, core_s1)]

    GS = TPG * P
    SMAX = ((max(cnts) + GS - 1) // GS) * GS
    T = SMAX // P
    NG = T // TPG

    def win_ok(gs):
        for k in range(NCORES):
            s0, cnt = core_s0[k], cnts[k]
            for g0 in range(0, cnt, gs):
                lo = seg[s0 + g0]
                hi = seg[s0 + min(g0 + gs, cnt) - 1]
                if hi - lo + 1 > P:
                    return False
        return True

    assert win_ok(GS), (
        "128-bag window does not cover a 512-sentence group; "
        "scope distribution far from expected"
    )

    X16 = np.asarray(X, dtype=np.float16)
    cons_np = np.ascontiguousarray(np.asarray(Constraints, dtype=np.float16))
    wt_np = np.ascontiguousarray(np.asarray(W, dtype=np.float16).T)  # [C, R]

    in_maps = []
    host_meta = []
    for k in range(NCORES):
        s0, s1, cnt = core_s0[k], core_s1[k], cnts[k]
        b0 = core_b0[k]

        xpad = np.zeros((SMAX, C), dtype=np.float16)
        xpad[:cnt] = X16[s0:s1]

        jj = np.arange(cnt)
        tt = jj // P
        pp = jj % P
        segl = seg[s0:s1] - b0                               # local bag ids

        wlo = np.zeros(NG, dtype=np.int64)                   # window base bag
        whi = np.full(NG, -1, dtype=np.int64)                # last real bag
        for g in range(NG):
            g0 = g * GS
            if g0 >= cnt:
                wlo[g] = 0
                whi[g] = -1
            else:
                wlo[g] = segl[g0]
                whi[g] = segl[min(g0 + GS, cnt) - 1]

        segloc = np.full((P, T), -1.0, dtype=np.float32)     # [p, t]
        segloc[pp, tt] = (segl - wlo[tt // TPG]).astype(np.float32)

        # batched per-group one-hots
        ohrel = np.zeros((NG, R, TPG * P), dtype=np.float16)
        ohrel[tt // TPG, rel_s[s0:s1], (tt % TPG) * P + pp] = 1.0

        ohexp = np.zeros((NG, P, TPG * P), dtype=np.float32)
        ohexp[tt // TPG, segl - wlo[tt // TPG], (tt % TPG) * P + pp] = 1.0
        ohexp = ohexp.astype(ml_dtypes.bfloat16)

        gidx = np.full((P, NG), SMAX, dtype=np.int32)        # -> zero tail
        glen = np.zeros((P, NG), dtype=np.float32)
        pr = np.arange(P)
        for g in range(NG):
            if whi[g] < 0:
                continue
            nreal = int(whi[g] - wlo[g]) + 1
            gb = b0 + wlo[g] + pr[:nreal]                    # global bag ids
            gidx[:nreal, g] = (starts[gb] - s0).astype(np.int32)
            glen[:nreal, g] = lens[gb].astype(np.float32)

        # per-(bag,row-pos) additive softmax mask: 0 inside the bag,
        # <= -6e4 beyond its length; clamped to stay finite in fp16
        amask = np.minimum((glen[:, :, None] - 1.0
                            - np.arange(LW, dtype=np.float32)) * 6.0e4, 0.0)
        amask = np.maximum(amask, -6.0e4)
        amask = np.ascontiguousarray(
            amask.transpose(1, 0, 2)).astype(np.float16)     # [NG, P, LW]

        in_maps.append(
            dict(
                xpad=xpad,
                ohrel=ohrel,
                ohexp=ohexp,
                segloc=segloc,
                gidx=gidx,
                amask=amask,
                cons=cons_np,
                wt=wt_np,
            )
        )
        host_meta.append(dict(wlo=wlo, whi=whi, b0=b0))

    meta = dict(C=C, R=R, NB_LOC=NB_LOC, SMAX=SMAX, T=T, NG=NG, LW=LW)
    return meta, in_maps, host_meta


# ----------------------------------------------------------------------------
# Bass program
# ----------------------------------------------------------------------------

def _build(meta):
    C, R = meta["C"], meta["R"]
    SMAX, T, NG, LW = meta["SMAX"], meta["T"], meta["NG"], meta["LW"]
    SZ = SMAX + LW + P                                       # s_lin length
    NCH = C // P                                             # 8 c-chunks
    GW = TPG * P                                             # group width

    nc = bacc.Bacc("TRN2", target_bir_lowering=False, debug=False,
                   num_devices=NCORES)

    xpad = nc.dram_tensor("xpad", (SMAX, C), F16, kind="ExternalInput").ap()
    ohrel = nc.dram_tensor("ohrel", (NG, R, GW), F16, kind="ExternalInput").ap()
    ohexp = nc.dram_tensor("ohexp", (NG, P, GW), BF16, kind="ExternalInput").ap()
    segloc = nc.dram_tensor("segloc", (P, T), F32, kind="ExternalInput").ap()
    gidx = nc.dram_tensor("gidx", (P, NG), I32, kind="ExternalInput").ap()
    amask = nc.dram_tensor("amask", (NG, P, LW), F16, kind="ExternalInput").ap()
    cons = nc.dram_tensor("cons", (R, C), F16, kind="ExternalInput").ap()
    wt = nc.dram_tensor("wt", (C, R), F16, kind="ExternalInput").ap()

    s_lin = nc.dram_tensor("s_lin", (1, SZ), F32, kind="Internal").ap()
    outg = nc.dram_tensor("outg", (NG, P, R), F32, kind="ExternalOutput").ap()
    if DEBUG:
        dbg_s = nc.dram_tensor("dbg_s", (P, T), F32, kind="ExternalOutput").ap()
        dbg_e = nc.dram_tensor("dbg_e", (P, T), F32, kind="ExternalOutput").ap()
        dbg_m = nc.dram_tensor("dbg_m", (P, NG), F32, kind="ExternalOutput").ap()
        dbg_rw = nc.dram_tensor("dbg_rw", (P, NG), F32, kind="ExternalOutput").ap()
        dbg_bag = nc.dram_tensor("dbg_bag", (NG, P, NCH * P), F16,
                                 kind="ExternalOutput").ap()
        dbg_bagti = nc.dram_tensor("dbg_bagti", (T, P, NCH * P), F16,
                                   kind="ExternalOutput").ap()
        dbg_ohp = nc.dram_tensor("dbg_ohp", (T, P, P), F16,
                                 kind="ExternalOutput").ap()

    with tile.TileContext(nc) as tc:
        with ExitStack() as ctx:
            singles = ctx.enter_context(tc.tile_pool(name="singles", bufs=1))
            xin = ctx.enter_context(tc.tile_pool(name="xin", bufs=28))
            ohrp = ctx.enter_context(tc.tile_pool(name="ohr", bufs=3))
            ohep = ctx.enter_context(tc.tile_pool(name="ohe", bufs=8))
            junkp = ctx.enter_context(tc.tile_pool(name="junk", bufs=2))
            junk2p = ctx.enter_context(tc.tile_pool(name="junk2", bufs=2))
            ohpp = ctx.enter_context(tc.tile_pool(name="ohp", bufs=4))
            spp = ctx.enter_context(tc.tile_pool(name="spp", bufs=5))
            tiny = ctx.enter_context(tc.tile_pool(name="tiny", bufs=8))
            bagtp = ctx.enter_context(tc.tile_pool(name="bagt", bufs=2))
            clsp = ctx.enter_context(tc.tile_pool(name="clsp", bufs=2))

            # PSUM: conper halves 2 banks, bagT 2x2 banks, cls/mcol small x2
            conp_ps = ctx.enter_context(
                tc.tile_pool(name="conp_ps", bufs=2, space="PSUM"))
            bagt_ps = ctx.enter_context(
                tc.tile_pool(name="bagt_ps", bufs=1, space="PSUM"))
            small_ps = ctx.enter_context(
                tc.tile_pool(name="small_ps", bufs=2, space="PSUM"))

            # ---------------- persistent tiles ----------------
            cons_sb = singles.tile([R, C], F16)
            nc.sync.dma_start(out=cons_sb[:], in_=cons[:])
            wt_sb = singles.tile([P, NCH, R], F16)
            nc.sync.dma_start(
                out=wt_sb[:], in_=wt.rearrange("(k p) r -> p k r", p=P))
            segloc_sb = singles.tile([P, T], F32)
            nc.sync.dma_start(out=segloc_sb[:], in_=segloc[:])
            gidx_sb = singles.tile([P, NG], I32)
            nc.sync.dma_start(out=gidx_sb[:], in_=gidx[:])

            iota_sb = singles.tile([P, P], F16)
            nc.gpsimd.iota(iota_sb[:], [[1, P]], channel_multiplier=0,
                           allow_small_or_imprecise_dtypes=True)
            ident32 = singles.tile([P, P], F32)
            make_identity(nc, ident32[:])

            swp = ctx.enter_context(tc.tile_pool(name="swp", bufs=8))
            ewp = ctx.enter_context(tc.tile_pool(name="ewp", bufs=3))
            srowp = ctx.enter_context(tc.tile_pool(name="srow", bufs=2))
            mnegbf = singles.tile([P, NG], BF16)
            mnegr = singles.tile([P, NG], F32)
            rw = singles.tile([P, NG], F32)

            ztail = singles.tile([1, LW + P], F32)
            nc.vector.memset(ztail[:], 0.0)
            z_inst = nc.scalar.dma_start(out=s_lin[0:1, SMAX:SZ], in_=ztail[:])

            s_dma_insts = [[] for _ in range(NG)]

            # ---------------- phase 1: scores, one tile ----------------
            def phase1_tile(g, ti, grp):
                t = g * TPG + ti
                x_t = xin.tile([P, C], F16, tag="x")
                nc.sync.dma_start(out=x_t[:], in_=xpad[t * P:(t + 1) * P, :])

                ohr_g = grp["ohr"]
                conper = conp_ps.tile([P, C], F32, tag="conper")
                for h in range(2):
                    nc.tensor.matmul(
                        out=conper[:, h * 512:(h + 1) * 512],
                        lhsT=ohr_g[:, ti * P:(ti + 1) * P],
                        rhs=cons_sb[:, h * 512:(h + 1) * 512],
                        start=True, stop=True)

                # score dot on DVE (gpsimd cannot read PSUM)
                junk = junkp.tile([P, C], F16, tag="junk")
                nc.vector.scalar_tensor_tensor(
                    out=junk[:], in0=x_t[:], scalar=1.0, in1=conper[:],
                    op0=mybir.AluOpType.mult, op1=mybir.AluOpType.mult,
                    accum_out=grp["sw"][:, ti:ti + 1])
                if ti == TPG - 1:
                    grp["cp3"] = conper
                return x_t

            def phase1_head(g):
                # batched per-group one-hot loads
                ohr_g = ohrp.tile([R, GW], F16, tag="ohr")
                nc.scalar.dma_start(out=ohr_g[:], in_=ohrel[g, :, :])
                ohe_g = ohep.tile([P, GW], BF16, tag="ohe")
                nc.sync.dma_start(out=ohe_g[:], in_=ohexp[g, :, :])
                sw_g = swp.tile([P, TPG], F32, tag="sw", name="sw")
                return dict(ohr=ohr_g, ohe=ohe_g, sw=sw_g)

            def phase1_tail(g, grp):
                # s [P, 4] -> transpose -> [4, P] -> contiguous DRAM store
                # (512B per descriptor instead of a 512x4B scatter); the
                # transpose lands in the group's last conper PSUM tile,
                # which the dot has just finished reading
                srow_ps = grp["cp3"][0:TPG, 0:P]
                nc.tensor.transpose(
                    out=srow_ps, in_=grp["sw"][:], identity=ident32[:])
                srow = srowp.tile([TPG, P], F32, tag="srow")
                nc.scalar.copy(out=srow[:], in_=srow_ps)
                # SWDGE queue: keeps the s_lin WAR chain (store <-> gather)
                # off the sync queue so x-loads never stall behind it
                dst = s_lin[0:1, g * GW:(g + 1) * GW]
                di = nc.gpsimd.dma_start(
                    out=dst.rearrange("o (q x) -> (o q) x", q=TPG),
                    in_=srow[:])
                s_dma_insts[g].append(di)

            # ---------------- phase 2: per-bag max / denom ----------------
            p2state = {}

            def phase2_fetch(g):
                am = spp.tile([P, LW], F16, tag="am")
                nc.scalar.dma_start(out=am[:], in_=amask[g, :, :])
                sp = spp.tile([P, LW], F32, tag="sp")
                gi = nc.gpsimd.indirect_dma_start(
                    out=sp[:],
                    out_offset=None,
                    in_=s_lin[:],
                    in_offset=IndirectOffsetOnAxis(ap=gidx_sb[:, g:g + 1], axis=1),
                )
                for gg in (max(g - 1, 0), g, min(g + 1, NG - 1)):
                    for di in s_dma_insts[gg]:
                        add_dep_helper(gi.ins, di.ins, reason="s_lin RAW")
                add_dep_helper(gi.ins, z_inst.ins, reason="s_lin tail RAW")
                p2state[g] = (am, sp)

            def phase2_comp(g):
                am, sp = p2state.pop(g)
                spm = spp.tile([P, LW], F32, tag="spm")
                nc.gpsimd.tensor_tensor(
                    out=spm[:], in0=sp[:], in1=am[:], op=mybir.AluOpType.add)
                mtmp = tiny.tile([P, 1], F32, tag="mtmp")
                nc.vector.tensor_reduce(
                    out=mtmp[:], in_=spm[:], axis=mybir.AxisListType.X,
                    op=mybir.AluOpType.max, negate=True)
                nc.gpsimd.tensor_scalar(
                    out=mnegbf[:, g:g + 1], in0=mtmp[:], scalar1=300.0,
                    scalar2=None, op0=mybir.AluOpType.min)
                nc.scalar.copy(out=mnegr[:, g:g + 1], in_=mnegbf[:, g:g + 1])
                epad = spp.tile([P, LW], F32, tag="epad")
                den = tiny.tile([P, 1], F32, tag="den")
                nc.scalar.activation(
                    out=epad[:], in_=spm[:],
                    func=mybir.ActivationFunctionType.Exp,
                    bias=mnegr[:, g:g + 1], scale=1.0, accum_out=den[:])
                dene = tiny.tile([P, 1], F32, tag="dene")
                nc.vector.tensor_scalar_add(dene[:], den[:], 1e-30)
                nc.vector.reciprocal(out=rw[:, g:g + 1], in_=dene[:])

            # ---------------- phase 3: pooling + classifier ----------------
            def phase3_tile(g, ti, state, x_t, grp):
                t = g * TPG + ti
                ohe_g = grp["ohe"]
                if ti == 0:
                    state["mcls"] = small_ps.tile([P, R + TPG], F32,
                                                  tag="smallps", name="mcls")
                    state["bagT"] = bagt_ps.tile([P, NCH, P], F32,
                                                 tag="bagtps", name="bagT")
                    state["ew"] = ewp.tile([P, TPG], F32, tag="ew", name="ew")
                mcls = state["mcls"]
                # gather -m[bag] into sentence layout (N=1 matmul)
                nc.tensor.matmul(
                    out=mcls[:, R + ti:R + ti + 1],
                    lhsT=ohe_g[:, ti * P:(ti + 1) * P],
                    rhs=mnegbf[:, g:g + 1],
                    start=True, stop=True)
                nc.scalar.activation(
                    out=state["ew"][:, ti:ti + 1],
                    in_=mcls[:, R + ti:R + ti + 1],
                    func=mybir.ActivationFunctionType.Exp,
                    bias=grp["sw"][:, ti:ti + 1],
                    scale=1.0)
                ohp = ohpp.tile([P, P], F16, tag="ohp")
                nc.vector.tensor_scalar(
                    out=ohp[:],
                    in0=iota_sb[:],
                    scalar1=segloc_sb[:, t:t + 1],
                    scalar2=state["ew"][:, ti:ti + 1],
                    op0=mybir.AluOpType.is_equal,
                    op1=mybir.AluOpType.mult)
                if DEBUG:
                    nc.sync.dma_start(out=dbg_ohp[t, :, :], in_=ohp[:])
                # transposed pooling: bagT[c,w] += x_chunk^T @ ohp
                bagT = state["bagT"]
                for cch in range(NCH):
                    # start only on the first chunk of each 2KB PSUM bank:
                    # first_mm clears the whole bank's has_written, so a
                    # start on every chunk wipes sibling chunks' state
                    nc.tensor.matmul(
                        out=bagT[:, cch, :],
                        lhsT=x_t[:, cch * P:(cch + 1) * P],
                        rhs=ohp[:],
                        start=(ti == 0 and cch % 4 == 0),
                        stop=(ti == TPG - 1))
                if DEBUG:
                    snap = bagtp.tile([P, NCH, P], F16, tag="bagt")
                    nc.scalar.copy(out=snap[:], in_=bagT[:])
                    nc.sync.dma_start(
                        out=dbg_bagti[t, :, :].rearrange(
                            "p (k q) -> p k q", k=NCH),
                        in_=snap[:])

            def phase3_tail_a(g, state):
                bagT = state["bagT"]
                bagt = bagtp.tile([P, NCH, P], F16, tag="bagt")
                nc.scalar.copy(out=bagt[:], in_=bagT[:])
                if DEBUG:
                    nc.sync.dma_start(
                        out=dbg_bag[g, :, :].rearrange("p (k q) -> p k q", k=NCH),
                        in_=bagt[:])
                state["bagt"] = bagt

            def phase3_tail_b(g, state):
                mcls, bagt = state["mcls"], state["bagt"]
                clsps = mcls[:, 0:R]
                for cch in range(NCH):
                    nc.tensor.matmul(
                        out=clsps,
                        lhsT=bagt[:, cch, :],
                        rhs=wt_sb[:, cch, :],
                        start=(cch == 0), stop=(cch == NCH - 1))
                cls_sb = clsp.tile([P, R], F32, tag="cls")
                nc.scalar.activation(
                    out=cls_sb[:], in_=clsps,
                    func=mybir.ActivationFunctionType.Copy,
                    scale=rw[:, g:g + 1])
                nc.scalar.dma_start(out=outg[g, :, :], in_=cls_sb[:])

            # ---------------- pipeline ----------------
            LAG = 4 if NG > 4 else 2
            live = {}
            states = {}
            grps = {}
            tailq = []

            def run_p3_tile(g, ti):
                x_t = live[g][ti]
                states.setdefault(g, {})
                phase3_tile(g, ti, states[g], x_t, grps[g])

            def pop_tail():
                if tailq:
                    fn, gg = tailq.pop(0)
                    fn(gg, states[gg])

            # comp(gg) runs 3 loops after its fetch so the DRAM
            # store->gather->mask chain never blocks the DVE/scalar FIFOs
            grps[0] = phase1_head(0)
            for g in range(NG):
                if g + 1 < NG:
                    grps[g + 1] = phase1_head(g + 1)
                tiles = []
                for ti in range(TPG):
                    tiles.append(phase1_tile(g, ti, grps[g]))
                    live[g] = tiles
                    if ti == 2 and g >= 3:
                        phase2_comp(g - 3)
                    pop_tail()
                    if g >= LAG:
                        run_p3_tile(g - LAG, ti)
                phase1_tail(g, grps[g])
                if g >= LAG:
                    gg = g - LAG
                    tailq.append((phase3_tail_a, gg))
                    tailq.append((phase3_tail_b, gg))
                if g >= 1:
                    phase2_fetch(g - 1)
            phase2_fetch(NG - 1)
            for gg in range(max(0, NG - 3), NG):
                phase2_comp(gg)
            for g in range(max(0, NG - LAG), NG):
                for ti in range(TPG):
                    pop_tail()
                    run_p3_tile(g, ti)
                tailq.append((phase3_tail_a, g))
                tailq.append((phase3_tail_b, g))
            while tailq:
                pop_tail()
            states.clear()
            live.clear()
            grps.clear()
            if DEBUG:
                nc.sync.dma_start(out=dbg_rw[:], in_=rw[:])
                nc.scalar.copy(out=mnegr[:], in_=mnegbf[:])
                nc.sync.dma_start(out=dbg_m[:], in_=mnegr[:])

    nc.compile()
    return nc


_CACHE = {}


def _get_program(meta):
    key = tuple(sorted(meta.items()))
    if key not in _CACHE:
        _CACHE[key] = _build(meta)
    return _CACHE[key]


def kernel(X, Constraints, W, b, X_Scope, X_Rel):
    X = np.asarray(X)
    b_np = np.asarray(b, dtype=np.float32)
    meta, in_maps, host_meta = _prep(X, Constraints, W, b, X_Scope, X_Rel)
    nc = _get_program(meta)
    res = run_bass_kernel_spmd(nc, in_maps, core_ids=list(range(NCORES)))
    NB_LOC, NG, R = meta["NB_LOC"], meta["NG"], meta["R"]
    B = NB_LOC * NCORES
    out = np.zeros((B, R), dtype=np.float32)
    for k in range(NCORES):
        og = res.results[k]["outg"]                       # [NG, P, R]
        hm = host_meta[k]
        wlo, whi, b0 = hm["wlo"], hm["whi"], hm["b0"]
        for g in range(NG):
            if whi[g] < 0:
                continue
            nreal = int(whi[g] - wlo[g]) + 1
            out[b0 + wlo[g]:b0 + wlo[g] + nreal] += og[g, :nreal]
    return out + b_np[None, :]
